# revision 1
# baseline (speedup 1.0000x reference)
"""Trainium2 Bass kernel for nn_DecoderLayer_66408784331382.

Single transformer decoder layer (RMSNorm + GQA attention w/ RoPE + RMSNorm +
SwiGLU MLP), tensor-parallel over 8 NeuronCores:

  - per core: 4 of 32 Q heads, 1 of 8 KV heads, 1024 of 8192 MLP inter cols,
    matching row-shards of wo / w_down.
  - all on-device activations are kept transposed ([hid, tok] etc.) so that
    every matmul is transpose-free; the host supplies hidden_states.T.
  - RMS statistics use an ACT Square pass + ones-column matmul (partition
    reduction); softmax denominators come from a ones-column appended to V in
    the PV matmul; per-token scaling uses partition-stride-0 broadcast DMAs.
  - one on-device fp32 AllReduce joins attention output partials before the
    second RMSNorm; the final down-proj partials (+ x1/8 each) are summed on
    the host during unsharding.
  - attention path is float32r (full-rate fp32 matmuls); the MLP runs bf16.

kernel(**inputs) takes the FULL fp32 inputs of reference.setup_inputs() and
returns the FULL [1, 2048, 2048] fp32 output.
"""

import sys

if "/opt/trn_rl_repo" not in sys.path:
    sys.path.insert(0, "/opt/trn_rl_repo")

import numpy as np
import ml_dtypes

import concourse.bass as bass
import concourse.mybir as mybir
import concourse.tile as tile
from concourse import bacc
from concourse.bass_utils import run_bass_kernel_spmd

# ---- problem constants (hardcoded per contract) ----
N_CORES = 8
S = 2048
HID = 2048
HD = 64
NH = 32
INTER = 8192
EPS = 1e-6

QD = (NH // N_CORES) * HD        # 256 local q cols
INTER_LOC = INTER // N_CORES     # 1024
SCALE = 1.0 / np.sqrt(HD)

F32 = mybir.dt.float32
F32R = mybir.dt.float32r
BF16 = mybir.dt.bfloat16

P = 128
Q = 512      # phase-1 token quarter
C = 1024     # phase-4 token chunk
ARDT = mybir.dt.float16  # collective dtype
AF = mybir.ActivationFunctionType
ALU = mybir.AluOpType


def _bcast(ap, parts):
    """View a [1, N] AP as [parts, N] via partition-stride-0 (DMA broadcast)."""
    return bass.AP(tensor=ap.tensor, offset=ap.offset,
                   ap=[[0, parts]] + [list(p) for p in ap.ap[1:]])


def build():
    nc = bacc.Bacc("TRN2", target_bir_lowering=False, debug=False,
                   num_devices=N_CORES)

    hT_d = nc.dram_tensor("hT", [HID, S], F32R, kind="ExternalInput")
    sin4_d = nc.dram_tensor("sin4", [P, S], F32R, kind="ExternalInput")
    cos4_d = nc.dram_tensor("cos4", [P, S], F32R, kind="ExternalInput")
    wq_d = nc.dram_tensor("wq", [HID, QD], F32R, kind="ExternalInput")
    wkv_d = nc.dram_tensor("wkv", [HID, 2 * HD], F32R, kind="ExternalInput")
    wo_d = nc.dram_tensor("wo", [QD, HID], F32R, kind="ExternalInput")
    wg_d = nc.dram_tensor("wg", [HID, INTER_LOC], BF16, kind="ExternalInput")
    wu_d = nc.dram_tensor("wu", [HID, INTER_LOC], BF16, kind="ExternalInput")
    wd_d = nc.dram_tensor("wd", [INTER_LOC, HID], BF16, kind="ExternalInput")
    ident_d = nc.dram_tensor("ident", [P, P], F32R, kind="ExternalInput")
    masks_d = nc.dram_tensor("masks", [P, 4 * 512], F32R, kind="ExternalInput")
    outT_d = nc.dram_tensor("outT", [HID, S], F32, kind="ExternalOutput")

    with tile.TileContext(nc) as tc, nc.allow_low_precision(
            reason="float32r is fp32 bits; reciprocal outputs are fp32-width"):
        with (
            tc.tile_pool(name="const", bufs=1) as const,
            tc.tile_pool(name="dramp", bufs=1, space="DRAM") as dram,
        ):
            ones1 = const.tile([P, 1], F32R)
            eps1 = const.tile([P, 1], F32)
            nc.gpsimd.memset(eps1, EPS)
            # f32r memset fails the walrus ISA check; masks[:,0,511] is all-1.0
            nc.sync.dma_start(
                ones1, bass.AP(tensor=masks_d.tensor
                               if hasattr(masks_d, "tensor") else masks_d,
                               offset=511, ap=[[4 * 512, P], [0, 1]]))

            ar_in = [dram.tile([HID, C], ARDT, name=f"ar_in{i}",
                               tag=f"ar_in{i}") for i in range(2)]
            ar_out = [dram.tile([HID, C], ARDT, addr_space="Shared",
                                name=f"ar_out{i}", tag=f"ar_out{i}")
                      for i in range(2)]
            bc1_dram = dram.tile([4, Q], F32R)
            bc2_dram = dram.tile([2, 8, 512], F32R)
            bc4_dram = dram.tile([2, C], F32R)

            # ======== attention scope (phases 1-3 share these tensors) ======
            with tc.tile_pool(name="keep", bufs=1) as keep:
                sin4 = keep.tile([P, S], F32R)
                cos4 = keep.tile([P, S], F32R)
                ident = keep.tile([P, P], F32R)
                masks = keep.tile([P, 4, 512], F32R)
                nc.sync.dma_start(sin4, sin4_d[:, :])
                nc.sync.dma_start(cos4, cos4_d[:, :])
                nc.sync.dma_start(ident, ident_d[:, :])
                nc.sync.dma_start(
                    masks, masks_d[:, :].rearrange("p (t n) -> p t n", t=4))
                qT = [keep.tile([P, S], F32R, tag=f"qT{m}", name=f"qT{m}") for m in range(2)]
                kTdup = keep.tile([P, S], F32R, tag="kTdup")
                v_ones = keep.tile([P, 16, HD + 1], F32R, tag="v_ones")
                attnT = [keep.tile([P, S], F32R, tag=f"attnT{m}", name=f"attnT{m}")
                         for m in range(2)]
                nc.sync.dma_start(
                    v_ones[:, :, HD:HD + 1],
                    bass.AP(tensor=masks_d.tensor
                            if hasattr(masks_d, "tensor") else masks_d,
                            offset=511, ap=[[4 * 512, P], [0, 16], [0, 1]]))

                # ---- Phase 1: RMS1 + QKV + RoPE, per 512-token quarter ----
                with (
                    tc.tile_pool(name="p1w", bufs=1) as p1w,
                    tc.tile_pool(name="p1x", bufs=1) as p1x,
                    tc.tile_pool(name="p1s", bufs=1) as p1s,
                    tc.tile_pool(name="p1ps", bufs=2, space="PSUM") as p1ps,
                    tc.tile_pool(name="p1ps_s", bufs=1, space="PSUM") as p1pss,
                ):
                    wq_all = p1w.tile([P, 16, QD], F32R)
                    wkv_all = p1w.tile([P, 16, 2 * HD], F32R)
                    nc.scalar.dma_start(
                        wq_all, wq_d[:, :].rearrange("(t p) m -> p t m", p=P))
                    nc.scalar.dma_start(
                        wkv_all, wkv_d[:, :].rearrange("(t p) m -> p t m", p=P))
                    xn1 = p1x.tile([P, 16, Q], F32R, tag="xn1")

                    for q4 in range(4):
                        qc = slice(Q * q4, Q * (q4 + 1))
                        # RMS1 stats
                        ssq = p1pss.tile([1, Q], F32, tag="ssq")
                        for t4 in range(4):
                            nc.sync.dma_start(
                                xn1[:, 4 * t4:4 * (t4 + 1), :],
                                hT_d[512 * t4:512 * (t4 + 1), qc].rearrange(
                                    "(t p) m -> p t m", p=P))
                        for kt in range(16):
                            xt = xn1[:, kt, :]
                            sq = p1s.tile([P, Q], F32R, tag="sq", bufs=3)
                            nc.scalar.activation(sq, xt, AF.Square)
                            nc.tensor.matmul(ssq, ones1, sq,
                                             start=(kt == 0), stop=(kt == 15))
                        rms = p1s.tile([1, Q], F32R, tag="rms", bufs=2)
                        nc.scalar.activation(rms, ssq, AF.Sqrt,
                                             bias=eps1[0:1, :], scale=1.0 / HID)
                        inv = p1s.tile([1, Q], F32R, tag="inv", bufs=2)
                        nc.vector.reciprocal(inv, rms)
                        invb = p1s.tile([P, Q], F32R, tag="invb", bufs=2)
                        nc.sync.dma_start(bc1_dram[q4:q4 + 1, :], inv)
                        nc.sync.dma_start(invb, _bcast(bc1_dram[q4:q4 + 1, :], P))
                        for kt in range(16):
                            nc.vector.tensor_mul(xn1[:, kt, :],
                                                 xn1[:, kt, :], invb)

                        # QKV projections (transposed outputs)
                        q_ps = [p1ps.tile([P, Q], F32, tag=f"qps{m}", name=f"qps{m}")
                                for m in range(2)]
                        kv_ps = p1ps.tile([P, Q], F32, tag="kvps")
                        for kt in range(16):
                            st, sp = (kt == 0), (kt == 15)
                            for m in range(2):
                                nc.tensor.matmul(
                                    q_ps[m], wq_all[:, kt, P * m:P * (m + 1)],
                                    xn1[:, kt, :], start=st, stop=sp)
                            nc.tensor.matmul(kv_ps, wkv_all[:, kt, :],
                                             xn1[:, kt, :], start=st, stop=sp)

                        # RoPE eviction (sin4 rows carry the rotate-half
                        # sign: +sinT for x0, -sinT for x1 source rows):
                        # out = ps*cos + swap_half(ps)*sinA
                        for m in range(2):
                            s1 = p1s.tile([P, Q], F32R, tag="s1", bufs=2)
                            s2 = p1s.tile([P, Q], F32R, tag="s2", bufs=2)
                            nc.vector.tensor_mul(s1, q_ps[m], cos4[:, qc])
                            for b in range(2):
                                x0 = slice(64 * b, 64 * b + 32)
                                x1s = slice(64 * b + 32, 64 * b + 64)
                                nc.vector.tensor_mul(
                                    s2[x0, :], q_ps[m][x1s, :], sin4[x1s, qc])
                                nc.vector.tensor_mul(
                                    s2[x1s, :], q_ps[m][x0, :], sin4[x0, qc])
                            nc.vector.tensor_add(qT[m][:, qc], s1, s2)
                        # RoPE eviction: k, duplicated into rows 64:128
                        s1 = p1s.tile([64, Q], F32R, tag="s1k", bufs=2)
                        s2 = p1s.tile([64, Q], F32R, tag="s2k", bufs=2)
                        nc.vector.tensor_mul(s1, kv_ps[0:64, :], cos4[0:64, qc])
                        nc.vector.tensor_mul(
                            s2[0:32, :], kv_ps[32:64, :], sin4[32:64, qc])
                        nc.vector.tensor_mul(
                            s2[32:64, :], kv_ps[0:32, :], sin4[0:32, qc])
                        nc.vector.tensor_add(kTdup[0:64, qc], s1, s2)
                        nc.vector.tensor_copy(kTdup[64:128, qc], kTdup[0:64, qc])
                        # v: vT then PE-transpose into v_ones
                        vt = p1s.tile([64, Q], F32R, tag="vt", bufs=2)
                        nc.vector.tensor_copy(vt, kv_ps[64:128, :])
                        for j in range(4):
                            ktg = 4 * q4 + j
                            vtp = p1pss.tile([P, HD], F32R, tag="vtp")
                            nc.tensor.transpose(
                                vtp, vt[:, P * j:P * (j + 1)],
                                ident[0:64, 0:64])
                            nc.vector.tensor_copy(v_ones[:, ktg, 0:HD], vtp)

                # ---- Phases 2+3 interleaved per token half: attention
                #      for half h, o-proj for half h, AllReduce(h).  The
                #      attention work of half 1 overlaps AllReduce(0). ----
                with (
                    tc.tile_pool(name="p2pr", bufs=3) as p2pr,
                    tc.tile_pool(name="p2sm", bufs=2) as p2sm,
                    tc.tile_pool(name="p3w", bufs=1) as p3w,
                    tc.tile_pool(name="p3o", bufs=3) as p3o,
                    tc.tile_pool(name="p2ps", bufs=2, space="PSUM") as p2ps,
                    tc.tile_pool(name="p2pv", bufs=1, space="PSUM") as p2pv,
                    tc.tile_pool(name="p3ps", bufs=2, space="PSUM") as p3ps,
                ):
                    wo_all = p3w.tile([P, 2, HID], F32R)
                    nc.scalar.dma_start(
                        wo_all, wo_d[:, :].rearrange("(t p) m -> p t m", p=P))
                    for c3 in range(2):
                        for qc4 in range(2 * c3, 2 * c3 + 2):
                            for m in range(2):
                                qs = slice(512 * qc4, 512 * (qc4 + 1))
                                pv = [p2pv.tile([HD + 1, 512], F32,
                                                tag=f"pv{b}", name=f"pv{b}")
                                      for b in range(2)]
                                nkt = 4 * qc4 + 4
                                for kt in range(nkt):
                                    st, sp = (kt == 0), (kt == nkt - 1)
                                    for b in range(2):
                                        rows = slice(64 * b, 64 * (b + 1))
                                        sc = p2ps.tile([P, 512], F32,
                                                       tag=f"sc{b}")
                                        nc.tensor.matmul(
                                            sc,
                                            kTdup[rows, P * kt:P * (kt + 1)],
                                            qT[m][rows, qs],
                                            start=True, stop=True)
                                        pr = p2pr.tile([P, 512], F32R,
                                                       tag=f"pr{b}")
                                        nc.scalar.activation(
                                            pr, sc, AF.Exp, scale=float(SCALE))
                                        if kt >= 4 * qc4:
                                            nc.vector.tensor_mul(
                                                pr, pr,
                                                masks[:, kt - 4 * qc4, :])
                                        nc.tensor.matmul(
                                            pv[b], v_ones[:, kt, :], pr,
                                            start=st, stop=sp)
                                for b in range(2):
                                    rec = p2sm.tile([1, 512], F32R,
                                                    tag=f"rec{b}")
                                    nc.vector.reciprocal(
                                        rec, pv[b][HD:HD + 1, :])
                                    slot = bc2_dram[b:b + 1, 4 * m + qc4, :]
                                    nc.sync.dma_start(slot, rec)
                                    recb = p2sm.tile([64, 512], F32R,
                                                     tag=f"recb{b}")
                                    nc.sync.dma_start(recb, _bcast(slot, 64))
                                    nc.vector.tensor_mul(
                                        attnT[m][64 * b:64 * (b + 1), qs],
                                        pv[b][0:HD, :], recb)
                        # o-proj for this half -> ar_in[c3]
                        for hm in range(16):
                            osb = p3o.tile([P, C], ARDT, tag="osb")
                            for nq in range(2):
                                qc4 = 2 * c3 + nq
                                qs = slice(512 * qc4, 512 * (qc4 + 1))
                                ops = p3ps.tile([P, 512], F32, tag="ops")
                                for kt2 in range(2):
                                    nc.tensor.matmul(
                                        ops,
                                        wo_all[:, kt2, P * hm:P * (hm + 1)],
                                        attnT[kt2][:, qs],
                                        start=(kt2 == 0), stop=(kt2 == 1))
                                nc.scalar.copy(
                                    osb[:, 512 * nq:512 * (nq + 1)], ops)
                            nc.gpsimd.dma_start(
                                ar_in[c3][P * hm:P * (hm + 1), :], osb)
                        # AllReduce for this token half (overlaps what follows)
                        nc.gpsimd.collective_compute(
                            "AllReduce", ALU.add,
                            replica_groups=[list(range(N_CORES))],
                            ins=[ar_in[c3][:, :].opt()],
                            outs=[ar_out[c3][:, :].opt()])

            # ---- Phase 4: x1 + RMS2 + SwiGLU MLP, per 1024-token chunk ----
            with (
                tc.tile_pool(name="p4x", bufs=1) as p4x,
                tc.tile_pool(name="p4s", bufs=1) as p4s,
                tc.tile_pool(name="p4w", bufs=1) as p4w,
                tc.tile_pool(name="p4ps_s", bufs=1, space="PSUM") as p4pss,
                tc.tile_pool(name="p4ps_gu", bufs=1, space="PSUM") as p4gu,
                tc.tile_pool(name="p4ps_d", bufs=1, space="PSUM") as p4d,
            ):
                x1 = p4x.tile([P, 16, C], F32R, tag="x1")
                xn2 = p4x.tile([P, 16, C], BF16, tag="xn2")
                hmlp = p4x.tile([P, 8, C], BF16, tag="hmlp")
                for c2 in range(2):
                    cc = slice(C * c2, C * (c2 + 1))
                    # x1 = hidden + attn_out ; RMS2 stats
                    ssq2 = p4pss.tile([1, C], F32, tag="ssq2")
                    for kt in range(16):
                        rs = slice(P * kt, P * (kt + 1))
                        th = p4s.tile([P, C], F32R, tag="th", bufs=2)
                        ta = p4s.tile([P, C], ARDT, tag="ta", bufs=2)
                        nc.sync.dma_start(th, hT_d[rs, cc])
                        nc.sync.dma_start(ta, ar_out[c2][rs, :])
                        nc.vector.tensor_add(x1[:, kt, :], th, ta)
                        sq = p4s.tile([P, C], F32R, tag="sq2", bufs=2)
                        nc.scalar.activation(sq, x1[:, kt, :], AF.Square)
                        for n in range(2):
                            nc.tensor.matmul(
                                ssq2[:, 512 * n:512 * (n + 1)], ones1,
                                sq[:, 512 * n:512 * (n + 1)],
                                start=(kt == 0), stop=(kt == 15))
                    rms = p4s.tile([1, C], F32R, tag="rms2", bufs=2)
                    nc.scalar.activation(rms, ssq2, AF.Sqrt,
                                         bias=eps1[0:1, :], scale=1.0 / HID)
                    inv = p4s.tile([1, C], F32R, tag="inv2", bufs=2)
                    nc.vector.reciprocal(inv, rms)
                    invb = p4s.tile([P, C], F32R, tag="invb2", bufs=1)
                    nc.sync.dma_start(bc4_dram[c2:c2 + 1, :], inv)
                    nc.sync.dma_start(invb, _bcast(bc4_dram[c2:c2 + 1, :], P))
                    for kt in range(16):
                        nc.vector.tensor_mul(xn2[:, kt, :], x1[:, kt, :], invb)

                    # gate/up + silu*up (bf16)
                    for iq in range(8):
                        gps = p4gu.tile([P, C], F32, tag="g")
                        ups = p4gu.tile([P, C], F32, tag="u")
                        wg_t = p4w.tile([P, 16, P], BF16, tag="wgt", bufs=2)
                        wu_t = p4w.tile([P, 16, P], BF16, tag="wut", bufs=2)
                        nc.scalar.dma_start(
                            wg_t, wg_d[:, P * iq:P * (iq + 1)].rearrange(
                                "(t p) m -> p t m", p=P))
                        nc.scalar.dma_start(
                            wu_t, wu_d[:, P * iq:P * (iq + 1)].rearrange(
                                "(t p) m -> p t m", p=P))
                        for kt in range(16):
                            st, sp = (kt == 0), (kt == 15)
                            for n in range(2):
                                ns = slice(512 * n, 512 * (n + 1))
                                nc.tensor.matmul(gps[:, ns], wg_t[:, kt, :],
                                                 xn2[:, kt, ns],
                                                 start=st, stop=sp)
                                nc.tensor.matmul(ups[:, ns], wu_t[:, kt, :],
                                                 xn2[:, kt, ns],
                                                 start=st, stop=sp)
                        sg = p4s.tile([P, C], BF16, tag="sg", bufs=2)
                        nc.scalar.activation(sg, gps, AF.Silu)
                        nc.vector.tensor_mul(hmlp[:, iq, :], sg, ups)

                    # down proj + (x1/8) residual share -> outT
                    for hm in range(16):
                        dps = p4d.tile([P, C], F32, tag="d")
                        wd_t = p4w.tile([P, 8, P], BF16, tag="wdt", bufs=2)
                        nc.scalar.dma_start(
                            wd_t, wd_d[:, P * hm:P * (hm + 1)].rearrange(
                                "(t p) m -> p t m", p=P))
                        for kt8 in range(8):
                            for n in range(2):
                                ns = slice(512 * n, 512 * (n + 1))
                                nc.tensor.matmul(dps[:, ns], wd_t[:, kt8, :],
                                                 hmlp[:, kt8, ns],
                                                 start=(kt8 == 0),
                                                 stop=(kt8 == 7))
                        dsb = p4s.tile([P, C], F32, tag="dsb", bufs=2)
                        nc.vector.scalar_tensor_tensor(
                            dsb, x1[:, hm, :], 1.0 / N_CORES, dps,
                            op0=ALU.mult, op1=ALU.add)
                        nc.gpsimd.dma_start(outT_d[P * hm:P * (hm + 1), cc], dsb)

    nc.compile()
    return nc


_CACHE = {}


def _get_nc():
    if "nc" not in _CACHE:
        _CACHE["nc"] = build()
    return _CACHE["nc"]


def _prep_inputs(inputs):
    """Shard + preprocess full inputs into 8 per-core in_maps."""
    f = lambda k: np.asarray(inputs[k], dtype=np.float32)
    hidden = f("hidden_states")[0]                 # [S, HID]
    sin_t, cos_t = f("sin_table"), f("cos_table")  # [S, 32]
    ln1, ln2 = f("ln1_w"), f("ln2_w")
    wq = f("wq") * ln1[:, None]
    wk = f("wk") * ln1[:, None]
    wv = f("wv") * ln1[:, None]
    wo = f("wo")
    wg = (f("w_gate") * ln2[:, None]).astype(ml_dtypes.bfloat16)
    wu = (f("w_up") * ln2[:, None]).astype(ml_dtypes.bfloat16)
    wd = f("w_down").astype(ml_dtypes.bfloat16)

    hT = np.ascontiguousarray(hidden.T)
    # rows per 64-block: [+sinT (x0 source); -sinT (x1 source)]
    sin4 = np.ascontiguousarray(
        np.tile(np.concatenate([sin_t.T, -sin_t.T], axis=0), (2, 1)))
    cos4 = np.ascontiguousarray(np.tile(cos_t.T, (4, 1)))
    ident = np.eye(P, dtype=np.float32)
    rr = np.arange(P)[:, None]
    cols = np.arange(512)[None, :]
    masks = np.concatenate(
        [(rr + 128 * t <= cols).astype(np.float32) for t in range(4)],
        axis=1)

    in_maps = []
    for c in range(N_CORES):
        qs = slice(QD * c, QD * (c + 1))
        ks = slice(HD * c, HD * (c + 1))
        isl = slice(INTER_LOC * c, INTER_LOC * (c + 1))
        in_maps.append({
            "hT": hT,
            "sin4": sin4,
            "cos4": cos4,
            "wq": np.ascontiguousarray(wq[:, qs]),
            "wkv": np.ascontiguousarray(
                np.concatenate([wk[:, ks], wv[:, ks]], axis=1)),
            "wo": np.ascontiguousarray(wo[qs, :]),
            "wg": np.ascontiguousarray(wg[:, isl]),
            "wu": np.ascontiguousarray(wu[:, isl]),
            "wd": np.ascontiguousarray(wd[isl, :]),
            "ident": ident,
            "masks": masks,
        })
    return in_maps


def kernel(**inputs):
    nc = _get_nc()
    in_maps = _prep_inputs(inputs)
    res = run_bass_kernel_spmd(nc, in_maps, core_ids=list(range(N_CORES)))
    acc = np.zeros((HID, S), dtype=np.float32)
    for c in range(N_CORES):
        acc += res.results[c]["outT"]
    return np.ascontiguousarray(acc.T)[None, :, :]



# revision 4
# speedup vs baseline: 1.2305x; 1.2305x over previous
"""Trainium2 Bass kernel for nn_DecoderLayer_66408784331382.

Single transformer decoder layer (RMSNorm + GQA attention w/ RoPE + RMSNorm +
SwiGLU MLP), tensor-parallel over 8 NeuronCores:

  - per core: 4 of 32 Q heads, 1 of 8 KV heads, 1024 of 8192 MLP inter cols,
    matching row-shards of wo / w_down.
  - everything runs in fp16 (full-rate PE matmuls, half the DMA bytes);
    PSUM accumulation stays fp32.
  - the RMS-norm per-token scale is folded into the RoPE eviction (cos/sin
    premultiplied by 1/rms) and the V eviction, so QKV matmuls consume the
    RAW hidden states - no separate normalize pass on the critical path.
  - attention inner loop is software-pipelined: PV matmuls of k-tile t are
    issued after the score matmuls of k-tile t+1, so the PE never waits on
    the scalar-engine exp.
  - AllReduce of the o-proj partials is split into 4 per-512-token chunks,
    each dispatched as soon as that token quarter's o-proj is done; the MLP
    consumes them chunk-by-chunk (stats for chunk q issued before the MLP of
    chunk q-1 so the rms roundtrip hides under matmuls).
  - final down-proj partials (+ x1/8 each) are summed on the host.

kernel(**inputs) takes the FULL fp32 inputs of reference.setup_inputs() and
returns the FULL [1, 2048, 2048] fp32 output.
"""

import sys

if "/opt/trn_rl_repo" not in sys.path:
    sys.path.insert(0, "/opt/trn_rl_repo")

import numpy as np

import concourse.bass as bass
import concourse.mybir as mybir
import concourse.tile as tile
from concourse import bacc
from concourse.bass_utils import run_bass_kernel_spmd

# ---- problem constants (hardcoded per contract) ----
N_CORES = 8
S = 2048
HID = 2048
HD = 64
NH = 32
INTER = 8192
EPS = 1e-6

QD = (NH // N_CORES) * HD        # 256 local q cols
INTER_LOC = INTER // N_CORES     # 1024
SCALE = 1.0 / np.sqrt(HD)

F32 = mybir.dt.float32
F16 = mybir.dt.float16

P = 128
Q = 512      # token quarter (attention / AR / MLP chunk)
AF = mybir.ActivationFunctionType
ALU = mybir.AluOpType


def _bcast(ap, parts):
    """View a [1, N] AP as [parts, N] via partition-stride-0 (DMA broadcast)."""
    return bass.AP(tensor=ap.tensor, offset=ap.offset,
                   ap=[[0, parts]] + [list(p) for p in ap.ap[1:]])


def build():
    nc = bacc.Bacc("TRN2", target_bir_lowering=False, debug=False,
                   num_devices=N_CORES)

    hT_d = nc.dram_tensor("hT", [HID, S], F16, kind="ExternalInput")
    sin4_d = nc.dram_tensor("sin4", [P, S], F16, kind="ExternalInput")
    cos4_d = nc.dram_tensor("cos4", [P, S], F16, kind="ExternalInput")
    wq_d = nc.dram_tensor("wq", [HID, QD], F16, kind="ExternalInput")
    wkv_d = nc.dram_tensor("wkv", [HID, 2 * HD], F16, kind="ExternalInput")
    wo_d = nc.dram_tensor("wo", [QD, HID], F16, kind="ExternalInput")
    wg_d = nc.dram_tensor("wg", [HID, INTER_LOC], F16, kind="ExternalInput")
    wu_d = nc.dram_tensor("wu", [HID, INTER_LOC], F16, kind="ExternalInput")
    wd_d = nc.dram_tensor("wd", [INTER_LOC, HID], F16, kind="ExternalInput")
    ident_d = nc.dram_tensor("ident", [P, P], F16, kind="ExternalInput")
    masks_d = nc.dram_tensor("masks", [P, 4 * Q], F16, kind="ExternalInput")
    outT_d = nc.dram_tensor("outT", [HID, S], F16, kind="ExternalOutput")

    mdt = masks_d.tensor if hasattr(masks_d, "tensor") else masks_d

    with tile.TileContext(nc) as tc, nc.allow_low_precision(
            reason="fp16 activations; tolerance is 2e-2"):
        with (
            tc.tile_pool(name="const", bufs=1) as const,
            tc.tile_pool(name="dramp", bufs=1, space="DRAM") as dram,
            tc.tile_pool(name="wmlp", bufs=1) as wmlp,
        ):
            ones1 = const.tile([P, 1], F16)
            eps1 = const.tile([P, 1], F32)
            nc.gpsimd.memset(eps1, EPS)
            # masks[:, 511] is all-1.0
            nc.sync.dma_start(
                ones1, bass.AP(tensor=mdt, offset=511, ap=[[4 * Q, P], [0, 1]]))

            ar_in = [dram.tile([HID, Q], F16, name=f"ar_in{i}",
                               tag=f"ar_in{i}") for i in range(4)]
            ar_out = [dram.tile([HID, Q], F16, addr_space="Shared",
                                name=f"ar_out{i}", tag=f"ar_out{i}")
                      for i in range(4)]
            bc1_dram = dram.tile([4, Q], F16)
            bc2_dram = dram.tile([2, 8, Q], F16)
            bc4_dram = dram.tile([4, Q], F16)

            # MLP gate/up weights: resident in SBUF for the whole kernel
            wg_all = wmlp.tile([P, 16, INTER_LOC], F16)
            wu_all = wmlp.tile([P, 16, INTER_LOC], F16)

            # ======== phases 1-3: RMS1+QKV+RoPE, attention, o-proj+AR ======
            with tc.tile_pool(name="keep", bufs=1) as keep:
                sin4 = keep.tile([P, S], F16)
                cos4 = keep.tile([P, S], F16)
                ident = keep.tile([P, P], F16)
                masks = keep.tile([P, 4, Q], F16)
                wq_all = keep.tile([P, 16, QD], F16)
                wkv_all = keep.tile([P, 16, 2 * HD], F16)
                wo_all = keep.tile([P, 2, HID], F16)
                qT = [keep.tile([P, S], F16, tag=f"qT{m}", name=f"qT{m}")
                      for m in range(2)]
                kTdup = keep.tile([P, S], F16, tag="kTdup")
                v_ones = keep.tile([P, 16, HD + 1], F16, tag="v_ones")
                attnT = [keep.tile([P, S], F16, tag=f"attnT{m}",
                                   name=f"attnT{m}") for m in range(2)]

                # weight/table loads: small attn weights first, then MLP
                nc.scalar.dma_start(
                    wq_all, wq_d[:, :].rearrange("(t p) m -> p t m", p=P))
                nc.scalar.dma_start(
                    wkv_all, wkv_d[:, :].rearrange("(t p) m -> p t m", p=P))
                nc.sync.dma_start(sin4, sin4_d[:, :])
                nc.sync.dma_start(cos4, cos4_d[:, :])
                nc.sync.dma_start(ident, ident_d[:, :])
                nc.sync.dma_start(
                    masks, masks_d[:, :].rearrange("p (t n) -> p t n", t=4))
                nc.sync.dma_start(
                    v_ones[:, :, HD:HD + 1],
                    bass.AP(tensor=mdt, offset=511,
                            ap=[[4 * Q, P], [0, 16], [0, 1]]))
                nc.scalar.dma_start(
                    wo_all, wo_d[:, :].rearrange("(t p) m -> p t m", p=P))
                nc.scalar.dma_start(
                    wg_all, wg_d[:, :].rearrange("(t p) m -> p t m", p=P))
                nc.scalar.dma_start(
                    wu_all, wu_d[:, :].rearrange("(t p) m -> p t m", p=P))

                # ---- Phase 1: QKV on raw x + RMS stats + fused-norm RoPE ----
                with (
                    tc.tile_pool(name="p1x", bufs=2) as p1x,
                    tc.tile_pool(name="p1s", bufs=1) as p1s,
                    tc.tile_pool(name="p1ps_q", bufs=2, space="PSUM") as p1q,
                    tc.tile_pool(name="p1ps_kv", bufs=2, space="PSUM") as p1kv,
                    tc.tile_pool(name="p1ps_s", bufs=1, space="PSUM") as p1pss,
                    tc.tile_pool(name="p1ps_t", bufs=1, space="PSUM") as p1pst,
                ):
                    vt_prev = None
                    for q4 in range(4):
                        qc = slice(Q * q4, Q * (q4 + 1))
                        xc = p1x.tile([P, 16, Q], F16, tag="xc")
                        for t4 in range(4):
                            nc.sync.dma_start(
                                xc[:, 4 * t4:4 * (t4 + 1), :],
                                hT_d[512 * t4:512 * (t4 + 1), qc].rearrange(
                                    "(t p) m -> p t m", p=P))
                        # QKV projections on RAW x (norm folded into eviction)
                        q_ps = [p1q.tile([P, Q], F32, tag=f"qps{m}",
                                         name=f"qps{m}") for m in range(2)]
                        kv_ps = p1kv.tile([P, Q], F32, tag="kvps")
                        for kt in range(16):
                            st, sp = (kt == 0), (kt == 15)
                            for m in range(2):
                                nc.tensor.matmul(
                                    q_ps[m], wq_all[:, kt, P * m:P * (m + 1)],
                                    xc[:, kt, :], start=st, stop=sp)
                            nc.tensor.matmul(kv_ps, wkv_all[:, kt, :],
                                             xc[:, kt, :], start=st, stop=sp)
                        # RMS1 stats (squares on scalar overlap QKV matmuls)
                        ssq = p1pss.tile([1, Q], F32, tag="ssq")
                        for kt in range(16):
                            sq = p1s.tile([P, Q], F16, tag="sq", bufs=3)
                            nc.scalar.activation(sq, xc[:, kt, :], AF.Square)
                            nc.tensor.matmul(ssq, ones1, sq,
                                             start=(kt == 0), stop=(kt == 15))
                        # v transposes of the PREVIOUS quarter (fills PE while
                        # this quarter's eviction chain completes on vec/dma)
                        if vt_prev is not None:
                            vtq, vt_t = vt_prev
                            for j in range(4):
                                vtp = p1pst.tile([P, HD], F16, tag="vtp")
                                nc.tensor.transpose(
                                    vtp, vt_t[:, P * j:P * (j + 1)],
                                    ident[0:64, 0:64])
                                nc.vector.tensor_copy(
                                    v_ones[:, 4 * vtq + j, 0:HD], vtp)
                        rms = p1s.tile([1, Q], F32, tag="rms", bufs=2)
                        nc.scalar.activation(rms, ssq, AF.Sqrt,
                                             bias=eps1[0:1, :], scale=1.0 / HID)
                        inv = p1s.tile([1, Q], F16, tag="inv", bufs=2)
                        nc.vector.reciprocal(inv, rms)
                        nc.sync.dma_start(bc1_dram[q4:q4 + 1, :], inv)
                        invb = p1s.tile([P, Q], F16, tag="invb", bufs=2)
                        nc.sync.dma_start(invb, _bcast(bc1_dram[q4:q4 + 1, :], P))
                        cosI = p1s.tile([P, Q], F16, tag="cosI", bufs=2)
                        sinI = p1s.tile([P, Q], F16, tag="sinI", bufs=2)
                        nc.vector.tensor_mul(cosI, cos4[:, qc], invb)
                        nc.vector.tensor_mul(sinI, sin4[:, qc], invb)
                        # RoPE eviction (sinI rows carry the rotate-half sign)
                        for m in range(2):
                            s1 = p1s.tile([P, Q], F16, tag="s1", bufs=2)
                            s2 = p1s.tile([P, Q], F16, tag="s2", bufs=2)
                            nc.vector.tensor_mul(s1, q_ps[m], cosI)
                            for b in range(2):
                                x0 = slice(64 * b, 64 * b + 32)
                                x1s = slice(64 * b + 32, 64 * b + 64)
                                nc.vector.tensor_mul(
                                    s2[x0, :], q_ps[m][x1s, :], sinI[x1s, :])
                                nc.vector.tensor_mul(
                                    s2[x1s, :], q_ps[m][x0, :], sinI[x0, :])
                            nc.vector.tensor_add(qT[m][:, qc], s1, s2)
                        # k rope (rows 0:64), duplicated into rows 64:128
                        s1k = p1s.tile([64, Q], F16, tag="s1k", bufs=2)
                        s2k = p1s.tile([64, Q], F16, tag="s2k", bufs=2)
                        nc.vector.tensor_mul(s1k, kv_ps[0:64, :], cosI[0:64, :])
                        nc.vector.tensor_mul(
                            s2k[0:32, :], kv_ps[32:64, :], sinI[32:64, :])
                        nc.vector.tensor_mul(
                            s2k[32:64, :], kv_ps[0:32, :], sinI[0:32, :])
                        nc.vector.tensor_add(kTdup[0:64, qc], s1k, s2k)
                        nc.vector.tensor_copy(kTdup[64:128, qc],
                                              kTdup[0:64, qc])
                        # v: scale by inv; transposed next quarter
                        vt = p1s.tile([64, Q], F16, tag="vt", bufs=2)
                        nc.vector.tensor_mul(vt, kv_ps[64:128, :],
                                             invb[0:64, :])
                        vt_prev = (q4, vt)
                    vtq, vt_t = vt_prev
                    for j in range(4):
                        vtp = p1pst.tile([P, HD], F16, tag="vtp")
                        nc.tensor.transpose(vtp, vt_t[:, P * j:P * (j + 1)],
                                            ident[0:64, 0:64])
                        nc.vector.tensor_copy(v_ones[:, 4 * vtq + j, 0:HD], vtp)

                # ---- Phases 2+3: attention + o-proj + AR, per 512-quarter.
                #      o-proj(q) is issued after attention(q+1) so the PE
                #      never waits on the softmax-denominator roundtrip. ----
                with (
                    tc.tile_pool(name="p2pr", bufs=3) as p2pr,
                    tc.tile_pool(name="p2sm", bufs=2) as p2sm,
                    tc.tile_pool(name="p3o", bufs=3) as p3o,
                    tc.tile_pool(name="p2ps", bufs=1, space="PSUM") as p2ps,
                    tc.tile_pool(name="p2pv", bufs=2, space="PSUM") as p2pv,
                    tc.tile_pool(name="p3ps", bufs=2, space="PSUM") as p3ps,
                ):
                    def attn(qc4):
                        qs = slice(Q * qc4, Q * (qc4 + 1))
                        nkt = 4 * qc4 + 4
                        for m in range(2):
                            pv = [p2pv.tile([HD + 1, 512], F32, tag=f"pv{b}",
                                            name=f"pv{b}") for b in range(2)]
                            prs = {}
                            # software pipeline: pv(kt-1) issued after sc(kt)
                            for kt in range(nkt + 1):
                                for b in range(2):
                                    if kt < nkt:
                                        rows = slice(64 * b, 64 * (b + 1))
                                        sc = p2ps.tile([P, 512], F32,
                                                       tag=f"sc{b}")
                                        nc.tensor.matmul(
                                            sc,
                                            kTdup[rows, P * kt:P * (kt + 1)],
                                            qT[m][rows, qs],
                                            start=True, stop=True)
                                        pr = p2pr.tile([P, 512], F16,
                                                       tag=f"pr{b}")
                                        nc.scalar.activation(
                                            pr, sc, AF.Exp, scale=float(SCALE))
                                        if kt >= 4 * qc4:
                                            nc.vector.tensor_mul(
                                                pr, pr,
                                                masks[:, kt - 4 * qc4, :])
                                        prs[(kt, b)] = pr
                                    if kt >= 1:
                                        nc.tensor.matmul(
                                            pv[b], v_ones[:, kt - 1, :],
                                            prs.pop((kt - 1, b)),
                                            start=(kt == 1), stop=(kt == nkt))
                            for b in range(2):
                                rec = p2sm.tile([1, 512], F16, tag=f"rec{b}")
                                nc.vector.reciprocal(rec, pv[b][HD:HD + 1, :])
                                slot = bc2_dram[b:b + 1, 2 * qc4 + m, :]
                                nc.sync.dma_start(slot, rec)
                                recb = p2sm.tile([64, 512], F16,
                                                 tag=f"recb{b}")
                                nc.sync.dma_start(recb, _bcast(slot, 64))
                                nc.vector.tensor_mul(
                                    attnT[m][64 * b:64 * (b + 1), qs],
                                    pv[b][0:HD, :], recb)

                    def oproj(qc4):
                        qs = slice(Q * qc4, Q * (qc4 + 1))
                        for hm in range(16):
                            ops = p3ps.tile([P, 512], F32, tag="ops")
                            for kt2 in range(2):
                                nc.tensor.matmul(
                                    ops,
                                    wo_all[:, kt2, P * hm:P * (hm + 1)],
                                    attnT[kt2][:, qs],
                                    start=(kt2 == 0), stop=(kt2 == 1))
                            osb = p3o.tile([P, 512], F16, tag="osb")
                            nc.scalar.copy(osb, ops)
                            nc.sync.dma_start(
                                ar_in[qc4][P * hm:P * (hm + 1), :], osb)
                        nc.gpsimd.collective_compute(
                            "AllReduce", ALU.add,
                            replica_groups=[list(range(N_CORES))],
                            ins=[ar_in[qc4][:, :].opt()],
                            outs=[ar_out[qc4][:, :].opt()])

                    attn(0)
                    attn(1)
                    oproj(0)
                    attn(2)
                    oproj(1)
                    attn(3)
                    oproj(2)
                    oproj(3)

            # ---- Phase 4: x1 + RMS2 + SwiGLU MLP, per 512-token chunk.
            #      stats(q) issued before mlp(q-1) to hide the rms chain. ----
            with (
                tc.tile_pool(name="p4w", bufs=1) as p4w,
                tc.tile_pool(name="p4x", bufs=2) as p4x,
                tc.tile_pool(name="p4ta", bufs=1) as p4ta,
                tc.tile_pool(name="p4xn", bufs=2) as p4xn,
                tc.tile_pool(name="p4h", bufs=1) as p4h,
                tc.tile_pool(name="p4s", bufs=1) as p4s,
                tc.tile_pool(name="p4ps_s", bufs=2, space="PSUM") as p4pss,
                tc.tile_pool(name="p4ps_gu", bufs=2, space="PSUM") as p4gu,
                tc.tile_pool(name="p4ps_d", bufs=2, space="PSUM") as p4d,
            ):
                wd_all = p4w.tile([P, 8, HID], F16)
                nc.scalar.dma_start(
                    wd_all, wd_d[:, :].rearrange("(t p) m -> p t m", p=P))

                def stats(q):
                    """x1 = x + attn (in-place in th tile), rms2, xn2."""
                    qc = slice(Q * q, Q * (q + 1))
                    th = p4x.tile([P, 16, Q], F16, tag="th")
                    ta = p4ta.tile([P, 16, Q], F16, tag="ta")
                    for t4 in range(4):
                        nc.sync.dma_start(
                            th[:, 4 * t4:4 * (t4 + 1), :],
                            hT_d[512 * t4:512 * (t4 + 1), qc].rearrange(
                                "(t p) m -> p t m", p=P))
                        nc.sync.dma_start(
                            ta[:, 4 * t4:4 * (t4 + 1), :],
                            ar_out[q][512 * t4:512 * (t4 + 1), :].rearrange(
                                "(t p) m -> p t m", p=P))
                    ssq2 = p4pss.tile([1, Q], F32, tag="ssq2")
                    for kt in range(16):
                        nc.vector.tensor_add(th[:, kt, :], th[:, kt, :],
                                             ta[:, kt, :])
                        sq = p4s.tile([P, Q], F16, tag="sq4", bufs=3)
                        nc.scalar.activation(sq, th[:, kt, :], AF.Square)
                        nc.tensor.matmul(ssq2, ones1, sq,
                                         start=(kt == 0), stop=(kt == 15))
                    rms = p4s.tile([1, Q], F32, tag="rms2", bufs=2)
                    nc.scalar.activation(rms, ssq2, AF.Sqrt,
                                         bias=eps1[0:1, :], scale=1.0 / HID)
                    inv = p4s.tile([1, Q], F16, tag="inv2", bufs=2)
                    nc.vector.reciprocal(inv, rms)
                    nc.sync.dma_start(bc4_dram[q:q + 1, :], inv)
                    invb = p4s.tile([P, Q], F16, tag="invb2", bufs=2)
                    nc.sync.dma_start(invb, _bcast(bc4_dram[q:q + 1, :], P))
                    xn2 = p4xn.tile([P, 16, Q], F16, tag="xn2")
                    for kt in range(16):
                        nc.vector.tensor_mul(xn2[:, kt, :], th[:, kt, :], invb)
                    return th, xn2

                def mlp(q, x1, xn2):
                    qc = slice(Q * q, Q * (q + 1))
                    hmlp = p4h.tile([P, 8, Q], F16, tag="hmlp")
                    for iq in range(8):
                        gps = p4gu.tile([P, Q], F32, tag="g")
                        ups = p4gu.tile([P, Q], F32, tag="u")
                        for kt in range(16):
                            st, sp = (kt == 0), (kt == 15)
                            nc.tensor.matmul(
                                gps, wg_all[:, kt, P * iq:P * (iq + 1)],
                                xn2[:, kt, :], start=st, stop=sp)
                            nc.tensor.matmul(
                                ups, wu_all[:, kt, P * iq:P * (iq + 1)],
                                xn2[:, kt, :], start=st, stop=sp)
                        sg = p4s.tile([P, Q], F16, tag="sg", bufs=2)
                        nc.scalar.activation(sg, gps, AF.Silu)
                        nc.vector.tensor_mul(hmlp[:, iq, :], sg, ups)
                    for hm in range(16):
                        dps = p4d.tile([P, Q], F32, tag="d")
                        for kt8 in range(8):
                            nc.tensor.matmul(
                                dps, wd_all[:, kt8, P * hm:P * (hm + 1)],
                                hmlp[:, kt8, :],
                                start=(kt8 == 0), stop=(kt8 == 7))
                        dsb = p4s.tile([P, Q], F16, tag="dsb", bufs=3)
                        nc.vector.scalar_tensor_tensor(
                            dsb, x1[:, hm, :], 1.0 / N_CORES, dps,
                            op0=ALU.mult, op1=ALU.add)
                        nc.scalar.dma_start(
                            outT_d[P * hm:P * (hm + 1), qc], dsb)

                pend = []
                for q in range(4):
                    pend.append((q,) + stats(q))
                    if q >= 1:
                        mlp(*pend.pop(0))
                while pend:
                    mlp(*pend.pop(0))

    nc.compile()
    return nc


_CACHE = {}


def _get_nc():
    if "nc" not in _CACHE:
        _CACHE["nc"] = build()
    return _CACHE["nc"]


def _prep_inputs(inputs):
    """Shard + preprocess full inputs into 8 per-core in_maps."""
    f = lambda k: np.asarray(inputs[k], dtype=np.float32)
    hidden = f("hidden_states")[0]                 # [S, HID]
    sin_t, cos_t = f("sin_table"), f("cos_table")  # [S, 32]
    ln1, ln2 = f("ln1_w"), f("ln2_w")
    F16 = np.float16
    wq = (f("wq") * ln1[:, None]).astype(F16)
    wk = (f("wk") * ln1[:, None]).astype(F16)
    wv = (f("wv") * ln1[:, None]).astype(F16)
    wo = f("wo").astype(F16)
    wg = (f("w_gate") * ln2[:, None]).astype(F16)
    wu = (f("w_up") * ln2[:, None]).astype(F16)
    wd = f("w_down").astype(F16)

    hT = np.ascontiguousarray(hidden.T).astype(F16)
    # rows per 64-block: [+sinT (x0 source); -sinT (x1 source)]
    sin4 = np.ascontiguousarray(
        np.tile(np.concatenate([sin_t.T, -sin_t.T], axis=0), (2, 1))).astype(F16)
    cos4 = np.ascontiguousarray(np.tile(cos_t.T, (4, 1))).astype(F16)
    ident = np.eye(P, dtype=F16)
    rr = np.arange(P)[:, None]
    cols = np.arange(512)[None, :]
    masks = np.concatenate(
        [(rr + 128 * t <= cols) for t in range(4)], axis=1).astype(F16)

    in_maps = []
    for c in range(N_CORES):
        qs = slice(QD * c, QD * (c + 1))
        ks = slice(HD * c, HD * (c + 1))
        isl = slice(INTER_LOC * c, INTER_LOC * (c + 1))
        in_maps.append({
            "hT": hT,
            "sin4": sin4,
            "cos4": cos4,
            "wq": np.ascontiguousarray(wq[:, qs]),
            "wkv": np.ascontiguousarray(
                np.concatenate([wk[:, ks], wv[:, ks]], axis=1)),
            "wo": np.ascontiguousarray(wo[qs, :]),
            "wg": np.ascontiguousarray(wg[:, isl]),
            "wu": np.ascontiguousarray(wu[:, isl]),
            "wd": np.ascontiguousarray(wd[isl, :]),
            "ident": ident,
            "masks": masks,
        })
    return in_maps


def kernel(**inputs):
    nc = _get_nc()
    in_maps = _prep_inputs(inputs)
    res = run_bass_kernel_spmd(nc, in_maps, core_ids=list(range(N_CORES)))
    acc = np.zeros((HID, S), dtype=np.float32)
    for c in range(N_CORES):
        acc += res.results[c]["outT"]
    return np.ascontiguousarray(acc.T)[None, :, :]


# revision 7
# speedup vs baseline: 1.2764x; 1.0373x over previous
"""Trainium2 Bass kernel for nn_DecoderLayer_66408784331382.

Single transformer decoder layer (RMSNorm + GQA attention w/ RoPE + RMSNorm +
SwiGLU MLP), tensor-parallel over 8 NeuronCores:

  - per core: 4 of 32 Q heads, 1 of 8 KV heads, 1024 of 8192 MLP inter cols,
    matching row-shards of wo / w_down.
  - everything runs in fp16 (full-rate PE matmuls, half the DMA bytes);
    PSUM accumulation stays fp32.
  - the RMS-norm per-token scale is folded into the RoPE eviction (cos/sin
    premultiplied by 1/rms) and the V eviction, so QKV matmuls consume the
    RAW hidden states - no separate normalize pass on the critical path.
  - attention: the two 64-row score matmuls of a k-tile land in one
    [128,1024] PSUM region and are exponentiated by a single scalar-engine
    ACT op (halves the per-op overhead; the ACT engine is the co-bottleneck
    of this phase). PV matmuls are software-pipelined one k-tile behind the
    score matmuls so the PE never waits on the exp.
  - elementwise squares (RMS stats) and the o-proj PSUM->f16 evictions run
    on the vector engine, keeping the scalar engine free for exp.
  - AllReduce of the o-proj partials is split into 4 per-512-token chunks,
    each dispatched as soon as that token quarter's o-proj is done; the MLP
    consumes them chunk-by-chunk (stats for chunk q issued before the MLP of
    chunk q-1 so the rms roundtrip hides under matmuls).
  - final down-proj partials (+ x1/8 each) are summed on the host.

kernel(**inputs) takes the FULL fp32 inputs of reference.setup_inputs() and
returns the FULL [1, 2048, 2048] fp32 output.
"""

import sys

if "/opt/trn_rl_repo" not in sys.path:
    sys.path.insert(0, "/opt/trn_rl_repo")

import numpy as np

import concourse.bass as bass
import concourse.mybir as mybir
import concourse.tile as tile
from concourse import bacc
from concourse.bass_utils import run_bass_kernel_spmd

# ---- problem constants (hardcoded per contract) ----
N_CORES = 8
S = 2048
HID = 2048
HD = 64
NH = 32
INTER = 8192
EPS = 1e-6

QD = (NH // N_CORES) * HD        # 256 local q cols
INTER_LOC = INTER // N_CORES     # 1024
SCALE = 1.0 / np.sqrt(HD)

F32 = mybir.dt.float32
F16 = mybir.dt.float16

P = 128
Q = 512      # token quarter (attention / AR / MLP chunk)
AF = mybir.ActivationFunctionType
ALU = mybir.AluOpType


def _bcast(ap, parts):
    """View a [1, N] AP as [parts, N] via partition-stride-0 (DMA broadcast)."""
    return bass.AP(tensor=ap.tensor, offset=ap.offset,
                   ap=[[0, parts]] + [list(p) for p in ap.ap[1:]])


def build():
    nc = bacc.Bacc("TRN2", target_bir_lowering=False, debug=False,
                   num_devices=N_CORES)

    hT_d = nc.dram_tensor("hT", [HID, S], F16, kind="ExternalInput")
    sin4_d = nc.dram_tensor("sin4", [P, S], F16, kind="ExternalInput")
    cos4_d = nc.dram_tensor("cos4", [P, S], F16, kind="ExternalInput")
    wq_d = nc.dram_tensor("wq", [HID, QD], F16, kind="ExternalInput")
    wkv_d = nc.dram_tensor("wkv", [HID, 2 * HD], F16, kind="ExternalInput")
    wo_d = nc.dram_tensor("wo", [QD, HID], F16, kind="ExternalInput")
    wg_d = nc.dram_tensor("wg", [HID, INTER_LOC], F16, kind="ExternalInput")
    wu_d = nc.dram_tensor("wu", [HID, INTER_LOC], F16, kind="ExternalInput")
    wd_d = nc.dram_tensor("wd", [INTER_LOC, HID], F16, kind="ExternalInput")
    ident_d = nc.dram_tensor("ident", [P, P], F16, kind="ExternalInput")
    masks_d = nc.dram_tensor("masks", [P, 4 * 2 * Q], F16, kind="ExternalInput")
    outT_d = nc.dram_tensor("outT", [HID, S], F16, kind="ExternalOutput")

    mdt = masks_d.tensor if hasattr(masks_d, "tensor") else masks_d
    MROW = 4 * 2 * Q   # masks row stride

    with tile.TileContext(nc) as tc, nc.allow_low_precision(
            reason="fp16 activations; tolerance is 2e-2"):
        with (
            tc.tile_pool(name="const", bufs=1) as const,
            tc.tile_pool(name="dramp", bufs=1, space="DRAM") as dram,
            tc.tile_pool(name="wmlp", bufs=1) as wmlp,
        ):
            ones1 = const.tile([P, 1], F16)
            eps1 = const.tile([P, 1], F32)
            nc.gpsimd.memset(eps1, EPS)
            # masks[:, 511] is all-1.0
            nc.sync.dma_start(
                ones1, bass.AP(tensor=mdt, offset=511, ap=[[MROW, P], [0, 1]]))

            ar_in = [dram.tile([HID, Q], F16, name=f"ar_in{i}",
                               tag=f"ar_in{i}") for i in range(4)]
            ar_out = [dram.tile([HID, Q], F16, addr_space="Shared",
                                name=f"ar_out{i}", tag=f"ar_out{i}")
                      for i in range(4)]
            bc1_dram = dram.tile([4, Q], F16)
            bc2_dram = dram.tile([2, 8, Q], F16)
            bc4_dram = dram.tile([4, Q], F16)

            # MLP gate/up weights: resident in SBUF (loads deferred below so
            # they don't starve the phase-1 critical DMAs)
            wg_all = wmlp.tile([P, 16, INTER_LOC], F16)
            wu_all = wmlp.tile([P, 16, INTER_LOC], F16)

            # ======== phases 1-3: RMS1+QKV+RoPE, attention, o-proj+AR ======
            with tc.tile_pool(name="keep", bufs=1) as keep:
                sin4 = keep.tile([P, S], F16)
                cos4 = keep.tile([P, S], F16)
                ident = keep.tile([P, P], F16)
                masks = keep.tile([P, 4, 2 * Q], F16)
                wq_all = keep.tile([P, 16, QD], F16)
                wkv_all = keep.tile([P, 16, 2 * HD], F16)
                wo_all = keep.tile([P, 2, HID], F16)
                qT = [keep.tile([P, S], F16, tag=f"qT{m}", name=f"qT{m}")
                      for m in range(2)]
                kTdup = keep.tile([P, S], F16, tag="kTdup")
                v_ones = keep.tile([P, 16, HD + 1], F16, tag="v_ones")
                attnT = [keep.tile([P, S], F16, tag=f"attnT{m}",
                                   name=f"attnT{m}") for m in range(2)]

                # urgent loads first: wq/wkv (scalar q); xc(q0) and the
                # tables are issued at the top of the phase-1 loop below.
                nc.scalar.dma_start(
                    wq_all, wq_d[:, :].rearrange("(t p) m -> p t m", p=P))
                nc.scalar.dma_start(
                    wkv_all, wkv_d[:, :].rearrange("(t p) m -> p t m", p=P))

                # ---- Phase 1: QKV on raw x + RMS stats + fused-norm RoPE ----
                with (
                    tc.tile_pool(name="p1x", bufs=2) as p1x,
                    tc.tile_pool(name="p1s", bufs=1) as p1s,
                    tc.tile_pool(name="p1ps_q", bufs=2, space="PSUM") as p1q,
                    tc.tile_pool(name="p1ps_kv", bufs=2, space="PSUM") as p1kv,
                    tc.tile_pool(name="p1ps_s", bufs=1, space="PSUM") as p1pss,
                    tc.tile_pool(name="p1ps_t", bufs=1, space="PSUM") as p1pst,
                ):
                    vt_prev = None
                    for q4 in range(4):
                        qc = slice(Q * q4, Q * (q4 + 1))
                        xc = p1x.tile([P, 16, Q], F16, tag="xc")
                        for t4 in range(4):
                            nc.sync.dma_start(
                                xc[:, 4 * t4:4 * (t4 + 1), :],
                                hT_d[512 * t4:512 * (t4 + 1), qc].rearrange(
                                    "(t p) m -> p t m", p=P))
                        # tables after xc(q0); big weights spread across
                        # quarters so they don't starve the phase-1 DMAs
                        if q4 == 0:
                            nc.sync.dma_start(sin4, sin4_d[:, :])
                            nc.sync.dma_start(cos4, cos4_d[:, :])
                            nc.sync.dma_start(ident, ident_d[:, :])
                            nc.sync.dma_start(
                                masks,
                                masks_d[:, :].rearrange("p (t n) -> p t n", t=4))
                            nc.sync.dma_start(
                                v_ones[:, :, HD:HD + 1],
                                bass.AP(tensor=mdt, offset=511,
                                        ap=[[MROW, P], [0, 16], [0, 1]]))
                        elif q4 == 1:
                            nc.scalar.dma_start(
                                wg_all,
                                wg_d[:, :].rearrange("(t p) m -> p t m", p=P))
                        elif q4 == 2:
                            nc.scalar.dma_start(
                                wu_all,
                                wu_d[:, :].rearrange("(t p) m -> p t m", p=P))
                        elif q4 == 3:
                            nc.scalar.dma_start(
                                wo_all,
                                wo_d[:, :].rearrange("(t p) m -> p t m", p=P))
                        # QKV projections on RAW x (norm folded into eviction)
                        q_ps = [p1q.tile([P, Q], F32, tag=f"qps{m}",
                                         name=f"qps{m}") for m in range(2)]
                        kv_ps = p1kv.tile([P, Q], F32, tag="kvps")
                        for kt in range(16):
                            st, sp = (kt == 0), (kt == 15)
                            for m in range(2):
                                nc.tensor.matmul(
                                    q_ps[m], wq_all[:, kt, P * m:P * (m + 1)],
                                    xc[:, kt, :], start=st, stop=sp)
                            nc.tensor.matmul(kv_ps, wkv_all[:, kt, :],
                                             xc[:, kt, :], start=st, stop=sp)
                        # RMS1 stats (squares on vector overlap QKV matmuls)
                        ssq = p1pss.tile([1, Q], F32, tag="ssq")
                        for kt in range(16):
                            sq = p1s.tile([P, Q], F16, tag="sq", bufs=3)
                            nc.vector.tensor_mul(sq, xc[:, kt, :], xc[:, kt, :])
                            nc.tensor.matmul(ssq, ones1, sq,
                                             start=(kt == 0), stop=(kt == 15))
                        # v transposes of the PREVIOUS quarter (fills PE while
                        # this quarter's eviction chain completes on vec/dma)
                        if vt_prev is not None:
                            vtq, vt_t = vt_prev
                            for j in range(4):
                                vtp = p1pst.tile([P, HD], F16, tag="vtp")
                                nc.tensor.transpose(
                                    vtp, vt_t[:, P * j:P * (j + 1)],
                                    ident[0:64, 0:64])
                                nc.vector.tensor_copy(
                                    v_ones[:, 4 * vtq + j, 0:HD], vtp)
                        rms = p1s.tile([1, Q], F32, tag="rms", bufs=2)
                        nc.scalar.activation(rms, ssq, AF.Sqrt,
                                             bias=eps1[0:1, :], scale=1.0 / HID)
                        inv = p1s.tile([1, Q], F16, tag="inv", bufs=2)
                        nc.vector.reciprocal(inv, rms)
                        nc.sync.dma_start(bc1_dram[q4:q4 + 1, :], inv)
                        invb = p1s.tile([P, Q], F16, tag="invb", bufs=2)
                        nc.sync.dma_start(invb, _bcast(bc1_dram[q4:q4 + 1, :], P))
                        cosI = p1s.tile([P, Q], F16, tag="cosI", bufs=2)
                        sinI = p1s.tile([P, Q], F16, tag="sinI", bufs=2)
                        nc.vector.tensor_mul(cosI, cos4[:, qc], invb)
                        nc.vector.tensor_mul(sinI, sin4[:, qc], invb)
                        # RoPE eviction (sinI rows carry the rotate-half sign)
                        for m in range(2):
                            s1 = p1s.tile([P, Q], F16, tag="s1", bufs=2)
                            s2 = p1s.tile([P, Q], F16, tag="s2", bufs=2)
                            nc.vector.tensor_mul(s1, q_ps[m], cosI)
                            for b in range(2):
                                x0 = slice(64 * b, 64 * b + 32)
                                x1s = slice(64 * b + 32, 64 * b + 64)
                                nc.vector.tensor_mul(
                                    s2[x0, :], q_ps[m][x1s, :], sinI[x1s, :])
                                nc.vector.tensor_mul(
                                    s2[x1s, :], q_ps[m][x0, :], sinI[x0, :])
                            nc.vector.tensor_add(qT[m][:, qc], s1, s2)
                        # k rope (rows 0:64), duplicated into rows 64:128
                        s1k = p1s.tile([64, Q], F16, tag="s1k", bufs=2)
                        s2k = p1s.tile([64, Q], F16, tag="s2k", bufs=2)
                        nc.vector.tensor_mul(s1k, kv_ps[0:64, :], cosI[0:64, :])
                        nc.vector.tensor_mul(
                            s2k[0:32, :], kv_ps[32:64, :], sinI[32:64, :])
                        nc.vector.tensor_mul(
                            s2k[32:64, :], kv_ps[0:32, :], sinI[0:32, :])
                        nc.vector.tensor_add(kTdup[0:64, qc], s1k, s2k)
                        nc.vector.tensor_copy(kTdup[64:128, qc],
                                              kTdup[0:64, qc])
                        # v: scale by inv; transposed next quarter
                        vt = p1s.tile([64, Q], F16, tag="vt", bufs=2)
                        nc.vector.tensor_mul(vt, kv_ps[64:128, :],
                                             invb[0:64, :])
                        vt_prev = (q4, vt)
                    vtq, vt_t = vt_prev
                    for j in range(4):
                        vtp = p1pst.tile([P, HD], F16, tag="vtp")
                        nc.tensor.transpose(vtp, vt_t[:, P * j:P * (j + 1)],
                                            ident[0:64, 0:64])
                        nc.vector.tensor_copy(v_ones[:, 4 * vtq + j, 0:HD], vtp)

                # ---- Phases 2+3: attention + o-proj + AR, per 512-quarter.
                #      o-proj(q) is issued after attention(q+1) so the PE
                #      never waits on the softmax-denominator roundtrip. ----
                with (
                    tc.tile_pool(name="p2pr", bufs=3) as p2pr,
                    tc.tile_pool(name="p2sm", bufs=2) as p2sm,
                    tc.tile_pool(name="p3o", bufs=3) as p3o,
                    tc.tile_pool(name="p2ps", bufs=2, space="PSUM") as p2ps,
                    tc.tile_pool(name="p2pv", bufs=1, space="PSUM") as p2pv,
                    tc.tile_pool(name="p3ps", bufs=2, space="PSUM") as p3ps,
                ):
                    def attn(qc4):
                        qs = slice(Q * qc4, Q * (qc4 + 1))
                        nkt = 4 * qc4 + 4
                        for m in range(2):
                            pv = [p2pv.tile([HD + 1, 512], F32, tag=f"pv{b}",
                                            name=f"pv{b}") for b in range(2)]
                            prs = {}
                            # pipeline: pv(kt-1) issued after scores/exp(kt)
                            for kt in range(nkt + 1):
                                if kt < nkt:
                                    sc2 = p2ps.tile([P, 2 * 512], F32,
                                                    tag="sc2")
                                    for b in range(2):
                                        rows = slice(64 * b, 64 * (b + 1))
                                        nc.tensor.matmul(
                                            sc2[:, 512 * b:512 * (b + 1)],
                                            kTdup[rows, P * kt:P * (kt + 1)],
                                            qT[m][rows, qs],
                                            start=True, stop=True)
                                    pr = p2pr.tile([P, 2 * 512], F16,
                                                   tag="pr")
                                    nc.scalar.activation(
                                        pr, sc2, AF.Exp, scale=float(SCALE))
                                    if kt >= 4 * qc4:
                                        nc.vector.tensor_mul(
                                            pr, pr, masks[:, kt - 4 * qc4, :])
                                    prs[kt] = pr
                                if kt >= 1:
                                    for b in range(2):
                                        nc.tensor.matmul(
                                            pv[b], v_ones[:, kt - 1, :],
                                            prs[kt - 1][:, 512 * b:512 * (b + 1)],
                                            start=(kt == 1), stop=(kt == nkt))
                                    del prs[kt - 1]
                            for b in range(2):
                                rec = p2sm.tile([1, 512], F16, tag=f"rec{b}")
                                nc.vector.reciprocal(rec, pv[b][HD:HD + 1, :])
                                slot = bc2_dram[b:b + 1, 2 * qc4 + m, :]
                                nc.sync.dma_start(slot, rec)
                                recb = p2sm.tile([64, 512], F16,
                                                 tag=f"recb{b}")
                                nc.sync.dma_start(recb, _bcast(slot, 64))
                                nc.vector.tensor_mul(
                                    attnT[m][64 * b:64 * (b + 1), qs],
                                    pv[b][0:HD, :], recb)

                    def oproj(qc4):
                        qs = slice(Q * qc4, Q * (qc4 + 1))
                        for hm in range(16):
                            ops = p3ps.tile([P, 512], F32, tag="ops")
                            for kt2 in range(2):
                                nc.tensor.matmul(
                                    ops,
                                    wo_all[:, kt2, P * hm:P * (hm + 1)],
                                    attnT[kt2][:, qs],
                                    start=(kt2 == 0), stop=(kt2 == 1))
                            osb = p3o.tile([P, 512], F16, tag="osb")
                            nc.vector.tensor_copy(osb, ops)
                            nc.sync.dma_start(
                                ar_in[qc4][P * hm:P * (hm + 1), :], osb)
                        nc.gpsimd.collective_compute(
                            "AllReduce", ALU.add,
                            replica_groups=[list(range(N_CORES))],
                            ins=[ar_in[qc4][:, :].opt()],
                            outs=[ar_out[qc4][:, :].opt()])

                    attn(0)
                    attn(1)
                    oproj(0)
                    attn(2)
                    oproj(1)
                    attn(3)
                    oproj(2)
                    oproj(3)

            # ---- Phase 4: x1 + RMS2 + SwiGLU MLP, per 512-token chunk.
            #      stats(q) issued before mlp(q-1) to hide the rms chain. ----
            with (
                tc.tile_pool(name="p4w", bufs=1) as p4w,
                tc.tile_pool(name="p4x", bufs=2) as p4x,
                tc.tile_pool(name="p4ta", bufs=1) as p4ta,
                tc.tile_pool(name="p4xn", bufs=2) as p4xn,
                tc.tile_pool(name="p4h", bufs=1) as p4h,
                tc.tile_pool(name="p4s", bufs=1) as p4s,
                tc.tile_pool(name="p4ps_s", bufs=2, space="PSUM") as p4pss,
                tc.tile_pool(name="p4ps_gu", bufs=2, space="PSUM") as p4gu,
                tc.tile_pool(name="p4ps_d", bufs=2, space="PSUM") as p4d,
            ):
                wd_all = p4w.tile([P, 8, HID], F16)
                nc.scalar.dma_start(
                    wd_all, wd_d[:, :].rearrange("(t p) m -> p t m", p=P))

                def stats(q):
                    """x1 = x + attn (in-place in th tile), rms2, xn2."""
                    qc = slice(Q * q, Q * (q + 1))
                    th = p4x.tile([P, 16, Q], F16, tag="th")
                    ta = p4ta.tile([P, 16, Q], F16, tag="ta")
                    for t4 in range(4):
                        nc.sync.dma_start(
                            th[:, 4 * t4:4 * (t4 + 1), :],
                            hT_d[512 * t4:512 * (t4 + 1), qc].rearrange(
                                "(t p) m -> p t m", p=P))
                        nc.sync.dma_start(
                            ta[:, 4 * t4:4 * (t4 + 1), :],
                            ar_out[q][512 * t4:512 * (t4 + 1), :].rearrange(
                                "(t p) m -> p t m", p=P))
                    ssq2 = p4pss.tile([1, Q], F32, tag="ssq2")
                    for kt in range(16):
                        nc.vector.tensor_add(th[:, kt, :], th[:, kt, :],
                                             ta[:, kt, :])
                        sq = p4s.tile([P, Q], F16, tag="sq4", bufs=3)
                        nc.vector.tensor_mul(sq, th[:, kt, :], th[:, kt, :])
                        nc.tensor.matmul(ssq2, ones1, sq,
                                         start=(kt == 0), stop=(kt == 15))
                    rms = p4s.tile([1, Q], F32, tag="rms2", bufs=2)
                    nc.scalar.activation(rms, ssq2, AF.Sqrt,
                                         bias=eps1[0:1, :], scale=1.0 / HID)
                    inv = p4s.tile([1, Q], F16, tag="inv2", bufs=2)
                    nc.vector.reciprocal(inv, rms)
                    nc.sync.dma_start(bc4_dram[q:q + 1, :], inv)
                    invb = p4s.tile([P, Q], F16, tag="invb2", bufs=2)
                    nc.sync.dma_start(invb, _bcast(bc4_dram[q:q + 1, :], P))
                    xn2 = p4xn.tile([P, 16, Q], F16, tag="xn2")
                    for kt in range(16):
                        nc.vector.tensor_mul(xn2[:, kt, :], th[:, kt, :], invb)
                    return th, xn2

                def mlp(q, x1, xn2):
                    qc = slice(Q * q, Q * (q + 1))
                    hmlp = p4h.tile([P, 8, Q], F16, tag="hmlp")
                    for iq in range(8):
                        gps = p4gu.tile([P, Q], F32, tag="g")
                        ups = p4gu.tile([P, Q], F32, tag="u")
                        for kt in range(16):
                            st, sp = (kt == 0), (kt == 15)
                            nc.tensor.matmul(
                                gps, wg_all[:, kt, P * iq:P * (iq + 1)],
                                xn2[:, kt, :], start=st, stop=sp)
                            nc.tensor.matmul(
                                ups, wu_all[:, kt, P * iq:P * (iq + 1)],
                                xn2[:, kt, :], start=st, stop=sp)
                        sg = p4s.tile([P, Q], F16, tag="sg", bufs=2)
                        nc.scalar.activation(sg, gps, AF.Silu)
                        nc.vector.tensor_mul(hmlp[:, iq, :], sg, ups)
                    for hm in range(16):
                        dps = p4d.tile([P, Q], F32, tag="d")
                        for kt8 in range(8):
                            nc.tensor.matmul(
                                dps, wd_all[:, kt8, P * hm:P * (hm + 1)],
                                hmlp[:, kt8, :],
                                start=(kt8 == 0), stop=(kt8 == 7))
                        dsb = p4s.tile([P, Q], F16, tag="dsb", bufs=3)
                        nc.vector.scalar_tensor_tensor(
                            dsb, x1[:, hm, :], 1.0 / N_CORES, dps,
                            op0=ALU.mult, op1=ALU.add)
                        nc.scalar.dma_start(
                            outT_d[P * hm:P * (hm + 1), qc], dsb)

                pend = []
                for q in range(4):
                    pend.append((q,) + stats(q))
                    if q >= 1:
                        mlp(*pend.pop(0))
                while pend:
                    mlp(*pend.pop(0))

    nc.compile()
    return nc


_CACHE = {}


def _get_nc():
    if "nc" not in _CACHE:
        _CACHE["nc"] = build()
    return _CACHE["nc"]


def _prep_inputs(inputs):
    """Shard + preprocess full inputs into 8 per-core in_maps."""
    f = lambda k: np.asarray(inputs[k], dtype=np.float32)
    hidden = f("hidden_states")[0]                 # [S, HID]
    sin_t, cos_t = f("sin_table"), f("cos_table")  # [S, 32]
    ln1, ln2 = f("ln1_w"), f("ln2_w")
    H = np.float16
    wq = (f("wq") * ln1[:, None]).astype(H)
    wk = (f("wk") * ln1[:, None]).astype(H)
    wv = (f("wv") * ln1[:, None]).astype(H)
    wo = f("wo").astype(H)
    wg = (f("w_gate") * ln2[:, None]).astype(H)
    wu = (f("w_up") * ln2[:, None]).astype(H)
    wd = f("w_down").astype(H)

    hT = np.ascontiguousarray(hidden.T).astype(H)
    # rows per 64-block: [+sinT (x0 source); -sinT (x1 source)]
    sin4 = np.ascontiguousarray(
        np.tile(np.concatenate([sin_t.T, -sin_t.T], axis=0), (2, 1))).astype(H)
    cos4 = np.ascontiguousarray(np.tile(cos_t.T, (4, 1))).astype(H)
    ident = np.eye(P, dtype=H)
    rr = np.arange(P)[:, None]
    cols = np.arange(512)[None, :]
    # per 128-key-tile causal mask, duplicated for the two 64-row q halves
    # that share one [128,1024] score tile
    masks = np.concatenate(
        [np.concatenate([(rr + 128 * t <= cols)] * 2, axis=1)
         for t in range(4)], axis=1).astype(H)

    in_maps = []
    for c in range(N_CORES):
        qs = slice(QD * c, QD * (c + 1))
        ks = slice(HD * c, HD * (c + 1))
        isl = slice(INTER_LOC * c, INTER_LOC * (c + 1))
        in_maps.append({
            "hT": hT,
            "sin4": sin4,
            "cos4": cos4,
            "wq": np.ascontiguousarray(wq[:, qs]),
            "wkv": np.ascontiguousarray(
                np.concatenate([wk[:, ks], wv[:, ks]], axis=1)),
            "wo": np.ascontiguousarray(wo[qs, :]),
            "wg": np.ascontiguousarray(wg[:, isl]),
            "wu": np.ascontiguousarray(wu[:, isl]),
            "wd": np.ascontiguousarray(wd[isl, :]),
            "ident": ident,
            "masks": masks,
        })
    return in_maps


def kernel(**inputs):
    nc = _get_nc()
    in_maps = _prep_inputs(inputs)
    res = run_bass_kernel_spmd(nc, in_maps, core_ids=list(range(N_CORES)))
    acc = np.zeros((HID, S), dtype=np.float32)
    for c in range(N_CORES):
        acc += res.results[c]["outT"]
    return np.ascontiguousarray(acc.T)[None, :, :]
